# revision 1
# baseline (speedup 1.0000x reference)
"""Multi-head self-attention (no mask) on 8 TRN2 NeuronCores.

Problem: B=2, T=2048, C=1024, H=16 heads, D=64.
    q/k/v = x @ W{q,k,v}.T + b;  att = softmax(q k^T / sqrt(D));
    y = att v;  out = y @ Wp.T + bp.

Sharding: core (b, g) with b in {0,1} batches x g in {0..3} head-groups of 4
heads.  Each core computes q/k/v for its 4 heads over the full sequence of its
batch, attention for those heads, and the partial output projection through its
256 columns of Wp.  The host sums the 4 partial projections per batch and adds
bp (a pure post-add).  No device collectives needed.

On-core dataflow (everything f32r = TF32-class rounding on the PE; PSUM
accumulation is fp32):
  - x^T and W^T tiles produced via PE-transpose (fp32 DMA-transpose unsupported).
  - q^T/k^T [256, T] channel-on-partition; v [T, 256] natural with a ones
    column per head (65-wide groups) so that the y'-matmul also produces the
    softmax denominators as PSUM row 64.
  - S^T tile = k_h^T.T @ q_h^T (K=64 matmul); P = exp(S/8) on ACT straight out
    of PSUM; y'_h accumulated over 16 key tiles with V' as stationary.
  - normalization: DVE reciprocal of row 64, GPSIMD partition-broadcast,
    DVE multiply; odd heads partition-shifted into the packed y^T tile via
    SBUF->SBUF DMA (DVE cannot shift partitions).
  - out_partial = y^T.T @ Wp^T slice, written natural-layout.
"""

import sys
from contextlib import ExitStack

import numpy as np

if "/opt/trn_rl_repo" not in sys.path:
    sys.path.insert(0, "/opt/trn_rl_repo")

import concourse.bass as bass
import concourse.mybir as mybir
import concourse.tile as tile
from concourse import bacc
from concourse.bass_utils import run_bass_kernel_spmd
from concourse.masks import make_identity

F32 = mybir.dt.float32
F32R = mybir.dt.float32r
Act = mybir.ActivationFunctionType

P = 128
B, C, HEADS, D = 2, 1024, 16, 64
GROUPS = 4            # head groups (tensor-parallel dimension)
HLOC = HEADS // GROUPS  # 4 heads per core
G = HLOC * D          # 256 channels per core
KT = C // P           # 8 contraction tiles
VW = D + 1            # v group width incl. ones column


def build(T=2048, mm_dt=F32R, qk_dt=F32R, attn_dt=F32R):
    """Build the per-core Bass program (identical on all 8 cores)."""
    TQ = 512            # query-chunk (matmul free dim)
    NTQ = T // TQ
    NS = T // P         # key tiles
    NXC = T // 256      # x-transpose chunks

    cast_needed = mm_dt != F32

    nc = bacc.Bacc("TRN2", target_bir_lowering=False, debug=False)
    x = nc.dram_tensor("x", [T, C], F32, kind="ExternalInput")
    wq = nc.dram_tensor("wq", [G, C], F32, kind="ExternalInput")
    wk = nc.dram_tensor("wk", [G, C], F32, kind="ExternalInput")
    wv = nc.dram_tensor("wv", [G, C], F32, kind="ExternalInput")
    wp = nc.dram_tensor("wp", [C, G], F32, kind="ExternalInput")
    bq = nc.dram_tensor("bq", [G], F32, kind="ExternalInput")
    bk = nc.dram_tensor("bk", [G], F32, kind="ExternalInput")
    bv = nc.dram_tensor("bv", [G], F32, kind="ExternalInput")
    out = nc.dram_tensor("out", [T, C], F32, kind="ExternalOutput")

    with tile.TileContext(nc) as tc, ExitStack() as ctx:
        persist = ctx.enter_context(tc.tile_pool(name="persist", bufs=1))

        ident = persist.tile([P, P], F32, tag="ident")
        make_identity(nc, ident[:])

        ones_row32 = persist.tile([1, P], F32, tag="ones_row32")
        nc.gpsimd.memset(ones_row32[:], 1.0)
        ones_row = persist.tile([1, P], mm_dt, tag="ones_row")
        nc.vector.tensor_copy(ones_row[:], ones_row32[:])

        ones4_32 = persist.tile([P, HLOC, 1], F32, tag="ones4_32")
        nc.gpsimd.memset(ones4_32[:], 1.0)
        ones4 = persist.tile([P, HLOC, 1], attn_dt, tag="ones4")
        nc.vector.tensor_copy(ones4[:], ones4_32[:])

        bq_pp = persist.tile([P, 2], F32, tag="bq_pp")
        bk_pp = persist.tile([P, 2], F32, tag="bk_pp")
        nc.sync.dma_start(bq_pp[:], bq[:].rearrange("(m p) -> p m", p=P))
        nc.sync.dma_start(bk_pp[:], bk[:].rearrange("(m p) -> p m", p=P))
        bv32 = persist.tile([1, G], F32, tag="bv32")
        nc.sync.dma_start(bv32[:], bv[None, :])
        bv_row = persist.tile([1, G], mm_dt, tag="bv_row")
        nc.vector.tensor_copy(bv_row[:], bv32[:])

        qT = persist.tile([P, 2, T], qk_dt, tag="qT")
        kT = persist.tile([P, 2, T], qk_dt, tag="kT")
        v_sb = persist.tile([P, NS, HLOC * VW], attn_dt, tag="v_sb")
        yT = persist.tile([P, 2, T], mm_dt, tag="yT")
        wpT = persist.tile([P, 2, C], mm_dt, tag="wpT")

        # ---------------- phase 1: transposes + QKV projections ----------------
        with (
            tc.tile_pool(name="xtp", bufs=1) as xtp,
            tc.tile_pool(name="wtp", bufs=1) as wtp,
            tc.tile_pool(name="stage", bufs=2) as stage,
            tc.tile_pool(name="ps1", bufs=2, space="PSUM") as ps1,
        ):
            xT = xtp.tile([P, KT, T], mm_dt, tag="xT")
            wqT = wtp.tile([P, KT, G], mm_dt, tag="wqT")
            wkT = wtp.tile([P, KT, G], mm_dt, tag="wkT")
            wvT = wtp.tile([P, KT, G], mm_dt, tag="wvT")

            # -- weight transposes: w [G, C] natural -> wT [C-tiles, G]
            for w_dram, wT in ((wq, wqT), (wk, wkT), (wv, wvT)):
                w_nat = stage.tile([P, 2, C], F32, tag="stg")
                nc.sync.dma_start(
                    w_nat[:], w_dram[:, :].rearrange("(a p) c -> p a c", p=P)
                )
                for ck in range(KT):
                    pt = ps1.tile([P, 2 * P], F32, tag="tr")
                    for j in range(2):
                        nc.tensor.transpose(
                            pt[:, j * P : (j + 1) * P],
                            w_nat[:, j, ck * P : (ck + 1) * P],
                            ident[:],
                        )
                    nc.vector.tensor_copy(wT[:, ck, :], pt[:])

            # -- wp transpose: wp [C, G] natural -> wpT [G-tiles, C]
            wp_nat = stage.tile([P, KT, G], F32, tag="stg")
            nc.sync.dma_start(
                wp_nat[:], wp[:, :].rearrange("(a p) g -> p a g", p=P)
            )
            for j in range(2):
                for ci in range(0, KT, 4):
                    pt4 = ps1.tile([P, 4 * P], F32, tag="tr")
                    for a in range(4):
                        nc.tensor.transpose(
                            pt4[:, a * P : (a + 1) * P],
                            wp_nat[:, ci + a, j * P : (j + 1) * P],
                            ident[:],
                        )
                    nc.vector.tensor_copy(
                        wpT[:, j, ci * P : (ci + 4) * P], pt4[:]
                    )

            # -- x transpose: x [T, C] -> xT [C-tiles, T], 256-row chunks
            for tch in range(NXC):
                x_nat = stage.tile([P, 2, C], F32, tag="stg")
                nc.sync.dma_start(
                    x_nat[:],
                    x[:, :].rearrange("(n a p) c -> n p a c", a=2, p=P)[tch],
                )
                for ck in range(KT):
                    pt = ps1.tile([P, 2 * P], F32, tag="tr")
                    for j in range(2):
                        nc.tensor.transpose(
                            pt[:, j * P : (j + 1) * P],
                            x_nat[:, j, ck * P : (ck + 1) * P],
                            ident[:],
                        )
                    nc.vector.tensor_copy(
                        xT[:, ck, 256 * tch : 256 * (tch + 1)], pt[:]
                    )

            # -- v projection, natural layout, ones column per head
            for s in range(NS):
                pv = ps1.tile([P, G], F32, tag="pv")
                for kk in range(KT):
                    nc.tensor.matmul(
                        pv[:],
                        xT[:, kk, s * P : (s + 1) * P],
                        wvT[:, kk, :],
                        start=(kk == 0),
                        stop=False,
                    )
                nc.tensor.matmul(
                    pv[:], ones_row[0:1, :], bv_row[0:1, :], start=False, stop=True
                )
                vs = v_sb[:, s, :].rearrange("p (h e) -> p h e", e=VW)
                nc.vector.tensor_copy(
                    vs[:, :, 0:D],
                    pv[:].rearrange("p (h d) -> p h d", d=D),
                )
                nc.vector.tensor_copy(vs[:, :, D : D + 1], ones4[:])

            # -- q^T / k^T projections: [G, T] channel-on-partition
            # (emitted after v, grouped by head-pair m so attention on pair 0
            # can start while pair 1 still projects)
            for m in range(2):
                for wT, bias_pp, dstT in ((wqT, bq_pp, qT), (wkT, bk_pp, kT)):
                    for tq in range(NTQ):
                        pq = ps1.tile([P, TQ], F32, tag="pq")
                        for kk in range(KT):
                            nc.tensor.matmul(
                                pq[:],
                                wT[:, kk, m * P : (m + 1) * P],
                                xT[:, kk, tq * TQ : (tq + 1) * TQ],
                                start=(kk == 0),
                                stop=(kk == KT - 1),
                            )
                        nc.scalar.activation(
                            dstT[:, m, tq * TQ : (tq + 1) * TQ],
                            pq[:],
                            Act.Identity,
                            bias=bias_pp[:, m : m + 1],
                            scale=1.0,
                        )

        # ---------------- phase 2: attention ----------------
        with (
            tc.tile_pool(name="ppool", bufs=4) as ppool,
            tc.tile_pool(name="npool", bufs=2) as npool,
            tc.tile_pool(name="sps", bufs=2, space="PSUM") as sps,
            tc.tile_pool(name="yps", bufs=2, space="PSUM") as yps,
        ):
            for pi in range(2):
                for tq in range(NTQ):
                    tqs = slice(tq * TQ, (tq + 1) * TQ)
                    py0 = yps.tile([VW, TQ], F32, tag="py0")
                    py1 = yps.tile([VW, TQ], F32, tag="py1")
                    py = [py0, py1]
                    for s in range(NS):
                        sp = sps.tile([P, 2 * TQ], F32, tag="sp")
                        for hh in range(2):
                            bp_ = 64 * hh
                            nc.tensor.matmul(
                                sp[:, hh * TQ : (hh + 1) * TQ],
                                kT[bp_ : bp_ + 64, pi, s * P : (s + 1) * P],
                                qT[bp_ : bp_ + 64, pi, tqs],
                                start=True,
                                stop=True,
                            )
                        pt = ppool.tile([P, 2 * TQ], attn_dt, tag="pt")
                        nc.scalar.activation(
                            pt[:], sp[:], Act.Exp, scale=1.0 / np.sqrt(D)
                        )
                        for hh in range(2):
                            h = 2 * pi + hh
                            nc.tensor.matmul(
                                py[hh][:],
                                v_sb[:, s, h * VW : (h + 1) * VW],
                                pt[:, hh * TQ : (hh + 1) * TQ],
                                start=(s == 0),
                                stop=(s == NS - 1),
                            )
                    # normalize: y_h / sums_h (sums in PSUM row 64)
                    for hh in range(2):
                        # sums row lives at PSUM partition 64; the custom-DVE
                        # reciprocal and gpsimd broadcast both require
                        # partition-0 inputs (they ignore AP partition
                        # offsets on HW), so: DVE copy (aligned) -> DMA
                        # partition-shift -> approx reciprocal at base 0.
                        srow = npool.tile([VW, TQ], F32, tag=f"srow{hh}")
                        nc.vector.tensor_copy(srow[D : D + 1, :], py[hh][D : D + 1, :])
                        srow0 = npool.tile([1, TQ], F32, tag=f"srow0{hh}")
                        nc.sync.dma_start(srow0[:], srow[D : D + 1, :])
                        recip0 = npool.tile([1, TQ], F32, tag=f"recip0{hh}")
                        nc.vector.reciprocal_approx_fast(recip0[0:1, :], srow0[0:1, :])
                        bcast = npool.tile([D, TQ], F32, tag=f"bcast{hh}")
                        nc.gpsimd.partition_broadcast(
                            bcast[:, :], recip0[0:1, :], channels=D
                        )
                        if hh == 0:
                            nc.vector.tensor_mul(
                                yT[0:D, pi, tqs], py[hh][0:D, :], bcast[:, :]
                            )
                        else:
                            y_tmp = npool.tile([D, TQ], mm_dt, tag="y_tmp")
                            nc.vector.tensor_mul(
                                y_tmp[:], py[hh][0:D, :], bcast[:, :]
                            )
                            nc.sync.dma_start(yT[D : 2 * D, pi, tqs], y_tmp[:])

        # ---------------- phase 3: output projection (partial) ----------------
        with (
            tc.tile_pool(name="ops2", bufs=3, space="PSUM") as ops2,
            tc.tile_pool(name="opool", bufs=3) as opool,
        ):
            for m in range(T // P):
                out_sb = opool.tile([P, C], F32, tag="osb")
                for n in range(2):
                    po = ops2.tile([P, 512], F32, tag="po")
                    for j in range(2):
                        nc.tensor.matmul(
                            po[:],
                            yT[:, j, m * P : (m + 1) * P],
                            wpT[:, j, n * 512 : (n + 1) * 512],
                            start=(j == 0),
                            stop=(j == 1),
                        )
                    nc.vector.tensor_copy(out_sb[:, n * 512 : (n + 1) * 512], po[:])
                nc.sync.dma_start(out[m * P : (m + 1) * P, :], out_sb[:])

    nc.finalize()
    return nc


_NC_CACHE = {}


def _get_nc(T=2048):
    if T not in _NC_CACHE:
        _NC_CACHE[T] = build(T=T)
    return _NC_CACHE[T]


def _make_in_maps(x, Wq, bq, Wk, bk, Wv, bv, Wp):
    in_maps = []
    for b in range(B):
        xb = np.ascontiguousarray(x[b], dtype=np.float32)
        for g in range(GROUPS):
            sl = slice(g * G, (g + 1) * G)
            in_maps.append(
                {
                    "x": xb,
                    "wq": np.ascontiguousarray(Wq[sl, :], dtype=np.float32),
                    "wk": np.ascontiguousarray(Wk[sl, :], dtype=np.float32),
                    "wv": np.ascontiguousarray(Wv[sl, :], dtype=np.float32),
                    "wp": np.ascontiguousarray(Wp[:, sl], dtype=np.float32),
                    "bq": np.ascontiguousarray(bq[sl], dtype=np.float32),
                    "bk": np.ascontiguousarray(bk[sl], dtype=np.float32),
                    "bv": np.ascontiguousarray(bv[sl], dtype=np.float32),
                }
            )
    return in_maps


def run(inputs, trace=False):
    """Run on 8 cores; returns (out [B,T,C] fp32, BassKernelResults)."""
    x = np.asarray(inputs["x"], dtype=np.float32)
    T = x.shape[1]
    in_maps = _make_in_maps(
        x,
        np.asarray(inputs["Wq"]), np.asarray(inputs["bq"]),
        np.asarray(inputs["Wk"]), np.asarray(inputs["bk"]),
        np.asarray(inputs["Wv"]), np.asarray(inputs["bv"]),
        np.asarray(inputs["Wp"]),
    )
    nc = _get_nc(T)
    res = run_bass_kernel_spmd(
        nc, in_maps, core_ids=list(range(B * GROUPS)), trace=trace
    )
    bp = np.asarray(inputs["bp"], dtype=np.float32)
    parts = [res.results[i]["out"] for i in range(B * GROUPS)]
    out = np.stack(
        [sum(parts[b * GROUPS : (b + 1) * GROUPS]) for b in range(B)]
    ) + bp[None, None, :]
    return out.astype(np.float32), res


def kernel(**inputs):
    out, _ = run(inputs, trace=False)
    return out



# revision 2
# speedup vs baseline: 1.0314x; 1.0314x over previous
"""Multi-head self-attention (no mask) on 8 TRN2 NeuronCores.

Problem: B=2, T=2048, C=1024, H=16 heads, D=64.
    q/k/v = x @ W{q,k,v}.T + b;  att = softmax(q k^T / sqrt(D));
    y = att v;  out = y @ Wp.T + bp.

Sharding: core (b, g) with b in {0,1} batches x g in {0..3} head-groups of 4
heads.  Each core computes q/k/v for its 4 heads over the full sequence of its
batch, attention for those heads, and the partial output projection through its
256 columns of Wp.  The host sums the 4 partial projections per batch and adds
bp.  No device collectives needed.

v2 design (from trace analysis of v1):
  - All transposes moved to the HOST: the kernel receives x^T [C,T],
    Wq/Wk/Wv^T [C,G] and Wp^T [G,C] pre-transposed, eliminating all 192
    PE-transpose instructions and their DVE drain copies.
  - q^T/k^T and the attention probabilities P are bf16: the row-tiled
    concurrent S-matmul pair (heads 2pi / 2pi+1 at PE rows 0-63 / 64-127)
    shares one moving-operand XBUS; fp32 operands made each stream run at
    half rate (500ns vs 230ns per 512-col matmul).
  - Loop order tq -> pi -> s.  The attention phase is ACT(exp)-bound
    (~1147ns per [128,1024] tile), so the PE has idle slots: the output
    projection for chunk tq-1 and the q^T projection for chunk tq+1 are
    interleaved one instruction per s-iteration into the attention stream.
  - Softmax denominators via a ones-column appended to each head's V
    (65-wide stationary), reciprocal on DVE, partition-broadcast on GPSIMD,
    normalize multiply on DVE (engines that are otherwise idle).
"""

import sys
from collections import deque
from contextlib import ExitStack

import numpy as np

if "/opt/trn_rl_repo" not in sys.path:
    sys.path.insert(0, "/opt/trn_rl_repo")

import concourse.bass as bass
import concourse.mybir as mybir
import concourse.tile as tile
from concourse import bacc
from concourse.bass_utils import run_bass_kernel_spmd

F32 = mybir.dt.float32
F32R = mybir.dt.float32r
BF16 = mybir.dt.bfloat16
Act = mybir.ActivationFunctionType

P = 128
B, C, HEADS, D = 2, 1024, 16, 64
GROUPS = 4              # head groups (tensor-parallel dimension)
HLOC = HEADS // GROUPS  # 4 heads per core
G = HLOC * D            # 256 channels per core
KT = C // P             # 8 contraction tiles
VW = D + 1              # v group width incl. ones column


def build(T=2048, qk_dt=BF16, attn_dt=BF16, mm_dt=F32R):
    TQ = 512            # query-chunk (matmul free dim)
    NTQ = T // TQ
    NS = T // P         # key tiles

    nc = bacc.Bacc("TRN2", target_bir_lowering=False, debug=False)
    xT = nc.dram_tensor("xT", [C, T], F32R, kind="ExternalInput")
    wqT = nc.dram_tensor("wqT", [C, G], F32R, kind="ExternalInput")
    wkT = nc.dram_tensor("wkT", [C, G], F32R, kind="ExternalInput")
    wvT = nc.dram_tensor("wvT", [C, G], F32R, kind="ExternalInput")
    wpT = nc.dram_tensor("wpT", [G, C], F32R, kind="ExternalInput")
    bq = nc.dram_tensor("bq", [G], F32, kind="ExternalInput")
    bk = nc.dram_tensor("bk", [G], F32, kind="ExternalInput")
    bv = nc.dram_tensor("bv", [G], F32, kind="ExternalInput")
    out = nc.dram_tensor("out", [T, C], F32, kind="ExternalOutput")

    with tile.TileContext(nc) as tc, ExitStack() as ctx:
        persist = ctx.enter_context(tc.tile_pool(name="persist", bufs=1))

        ones4 = persist.tile([P, HLOC, 1], attn_dt, tag="ones4")
        nc.gpsimd.memset(ones4[:], 1.0)

        bq_pp = persist.tile([P, 2], F32, tag="bq_pp")
        bk_pp = persist.tile([P, 2], F32, tag="bk_pp")
        nc.sync.dma_start(bq_pp[:], bq[:].rearrange("(m p) -> p m", p=P))
        nc.sync.dma_start(bk_pp[:], bk[:].rearrange("(m p) -> p m", p=P))
        bv_row = persist.tile([1, G], F32, tag="bv_row")
        nc.sync.dma_start(bv_row[:], bv[None, :])
        bv_bc = persist.tile([P, G], F32, tag="bv_bc")
        nc.gpsimd.partition_broadcast(bv_bc[:, :], bv_row[0:1, :], channels=P)

        # persistent SBUF operands
        x_sb = persist.tile([P, KT, T], F32R, tag="x_sb")
        wq_sb = persist.tile([P, KT, G], F32R, tag="wq_sb")
        wk_sb = persist.tile([P, KT, G], F32R, tag="wk_sb")
        wv_sb = persist.tile([P, KT, G], F32R, tag="wv_sb")
        wp_sb = persist.tile([P, 2, C], F32R, tag="wp_sb")

        nc.sync.dma_start(wk_sb[:], wkT[:, :].rearrange("(a p) g -> p a g", p=P))
        nc.sync.dma_start(wv_sb[:], wvT[:, :].rearrange("(a p) g -> p a g", p=P))
        nc.sync.dma_start(wq_sb[:], wqT[:, :].rearrange("(a p) g -> p a g", p=P))
        nc.sync.dma_start(wp_sb[:], wpT[:, :].rearrange("(j p) c -> p j c", p=P))
        for c in range(NTQ):
            cs = slice(c * TQ, (c + 1) * TQ)
            nc.sync.dma_start(
                x_sb[:, :, cs], xT[:, cs].rearrange("(a p) t -> p a t", p=P)
            )

        qT = persist.tile([P, 2, T], qk_dt, tag="qT")
        kT = persist.tile([P, 2, T], qk_dt, tag="kT")
        v_sb = persist.tile([P, NS, HLOC * VW], attn_dt, tag="v_sb")
        yT = persist.tile([P, 2, T], mm_dt, tag="yT")

        # ---------------- phase A: k/v/q0 projections ----------------
        with tc.tile_pool(name="pa", bufs=2, space="PSUM") as pa:
            for c in range(NTQ):
                cs = slice(c * TQ, (c + 1) * TQ)
                # k^T chunk c: [G, TQ] channel-on-partition
                for m in range(2):
                    pk = pa.tile([P, TQ], F32, tag="pk")
                    for kk in range(KT):
                        nc.tensor.matmul(
                            pk[:],
                            wk_sb[:, kk, m * P : (m + 1) * P],
                            x_sb[:, kk, cs],
                            start=(kk == 0),
                            stop=(kk == KT - 1),
                        )
                    nc.scalar.activation(
                        kT[:, m, cs], pk[:], Act.Identity,
                        bias=bk_pp[:, m : m + 1], scale=1.0,
                    )
                # v tiles for keys in chunk c, natural layout + ones column
                for s in range(4 * c, 4 * c + 4):
                    pv = pa.tile([P, G], F32, tag="pv")
                    for kk in range(KT):
                        nc.tensor.matmul(
                            pv[:],
                            x_sb[:, kk, s * P : (s + 1) * P],
                            wv_sb[:, kk, :],
                            start=(kk == 0),
                            stop=(kk == KT - 1),
                        )
                    vs = v_sb[:, s, :].rearrange("p (h e) -> p h e", e=VW)
                    nc.vector.tensor_tensor(
                        vs[:, :, 0:D],
                        pv[:].rearrange("p (h d) -> p h d", d=D),
                        bv_bc[:].rearrange("p (h d) -> p h d", d=D),
                        op=mybir.AluOpType.add,
                    )
                    nc.vector.tensor_copy(vs[:, :, D : D + 1], ones4[:])
            # q^T chunk 0
            for m in range(2):
                pq = pa.tile([P, TQ], F32, tag="pk")
                for kk in range(KT):
                    nc.tensor.matmul(
                        pq[:],
                        wq_sb[:, kk, m * P : (m + 1) * P],
                        x_sb[:, kk, 0:TQ],
                        start=(kk == 0),
                        stop=(kk == KT - 1),
                    )
                nc.scalar.activation(
                    qT[:, m, 0:TQ], pq[:], Act.Identity,
                    bias=bq_pp[:, m : m + 1], scale=1.0,
                )

        # ---------------- phase B: attention + interleaved proj ----------------
        with (
            tc.tile_pool(name="ptp", bufs=3) as ptp,
            tc.tile_pool(name="npool", bufs=2) as npool,
            tc.tile_pool(name="osb", bufs=2) as osb_pool,
            tc.tile_pool(name="sps", bufs=2, space="PSUM") as sps,
            tc.tile_pool(name="yps", bufs=1, space="PSUM") as yps,
            tc.tile_pool(name="xps", bufs=1, space="PSUM") as xps,
        ):
            def qnext_steps(tqn):
                """q^T projection for chunk tqn, one emission step at a time."""
                tqs = slice(tqn * TQ, (tqn + 1) * TQ)
                for m in range(2):
                    pq = xps.tile([P, TQ], F32, tag="pq")
                    for kk in range(KT):
                        yield lambda m=m, kk=kk, pq=pq, tqs=tqs: nc.tensor.matmul(
                            pq[:],
                            wq_sb[:, kk, m * P : (m + 1) * P],
                            x_sb[:, kk, tqs],
                            start=(kk == 0),
                            stop=(kk == KT - 1),
                        )
                    yield lambda m=m, pq=pq, tqs=tqs: nc.vector.tensor_scalar_add(
                        qT[:, m, tqs], pq[:], bq_pp[:, m : m + 1]
                    )

            def oproj_steps(tqp):
                """output projection for query chunk tqp (4 row-tiles)."""
                for mi in range(4 * tqp, 4 * tqp + 4):
                    ob = osb_pool.tile([P, C], F32, tag="ob")
                    for n in range(2):
                        po = xps.tile([P, 512], F32, tag="po")
                        for j in range(2):
                            yield lambda mi=mi, n=n, j=j, po=po: nc.tensor.matmul(
                                po[:],
                                yT[:, j, mi * P : (mi + 1) * P],
                                wp_sb[:, j, n * 512 : (n + 1) * 512],
                                start=(j == 0),
                                stop=(j == 1),
                            )
                        yield lambda n=n, po=po, ob=ob: nc.vector.tensor_copy(
                            ob[:, n * 512 : (n + 1) * 512], po[:]
                        )
                    yield lambda mi=mi, ob=ob: nc.sync.dma_start(
                        out[mi * P : (mi + 1) * P, :], ob[:]
                    )

            for tq in range(NTQ):
                tqs = slice(tq * TQ, (tq + 1) * TQ)
                extras = deque()
                if tq + 1 < NTQ:
                    extras.extend(qnext_steps(tq + 1))
                if tq > 0:
                    extras.extend(oproj_steps(tq - 1))
                for pi in range(2):
                    py0 = yps.tile([VW, TQ], F32, tag="py0")
                    py1 = yps.tile([VW, TQ], F32, tag="py1")
                    py = [py0, py1]
                    for s in range(NS):
                        sp = sps.tile([P, 2 * TQ], F32, tag="sp")
                        for hh in range(2):
                            bp_ = 64 * hh
                            nc.tensor.matmul(
                                sp[:, hh * TQ : (hh + 1) * TQ],
                                kT[bp_ : bp_ + 64, pi, s * P : (s + 1) * P],
                                qT[bp_ : bp_ + 64, pi, tqs],
                                start=True,
                                stop=True,
                            )
                        pt = ptp.tile([P, 2 * TQ], attn_dt, tag="pt")
                        nc.scalar.activation(
                            pt[:], sp[:], Act.Exp, scale=1.0 / np.sqrt(D)
                        )
                        for hh in range(2):
                            h = 2 * pi + hh
                            nc.tensor.matmul(
                                py[hh][:],
                                v_sb[:, s, h * VW : (h + 1) * VW],
                                pt[:, hh * TQ : (hh + 1) * TQ],
                                start=(s == 0),
                                stop=(s == NS - 1),
                            )
                        if extras:
                            extras.popleft()()
                    # normalize: y_h / sums_h (sums live in PSUM row 64; the
                    # DVE reciprocal and gpsimd broadcast need partition-0
                    # inputs, so: DVE copy (aligned) -> DMA partition-shift ->
                    # reciprocal at base 0 -> broadcast -> multiply)
                    for hh in range(2):
                        srow = npool.tile([VW, TQ], F32, tag=f"srow{hh}")
                        nc.vector.tensor_copy(srow[D : D + 1, :], py[hh][D : D + 1, :])
                        srow0 = npool.tile([1, TQ], F32, tag=f"srow0{hh}")
                        nc.sync.dma_start(srow0[:], srow[D : D + 1, :])
                        recip0 = npool.tile([1, TQ], F32, tag=f"recip0{hh}")
                        nc.vector.reciprocal_approx_fast(recip0[0:1, :], srow0[0:1, :])
                        bcast = npool.tile([D, TQ], F32, tag=f"bcast{hh}")
                        nc.gpsimd.partition_broadcast(
                            bcast[:, :], recip0[0:1, :], channels=D
                        )
                        if hh == 0:
                            nc.vector.tensor_mul(
                                yT[0:D, pi, tqs], py[hh][0:D, :], bcast[:, :]
                            )
                        else:
                            y_tmp = npool.tile([D, TQ], mm_dt, tag="y_tmp")
                            nc.vector.tensor_mul(
                                y_tmp[:], py[hh][0:D, :], bcast[:, :]
                            )
                            nc.sync.dma_start(yT[D : 2 * D, pi, tqs], y_tmp[:])
                while extras:
                    extras.popleft()()
            # output projection for the final chunk
            for step in oproj_steps(NTQ - 1):
                step()

    nc.finalize()
    return nc


_NC_CACHE = {}


def _get_nc(T=2048):
    if T not in _NC_CACHE:
        _NC_CACHE[T] = build(T=T)
    return _NC_CACHE[T]


def _make_in_maps(x, Wq, bq, Wk, bk, Wv, bv, Wp):
    in_maps = []
    for b in range(B):
        xT_b = np.ascontiguousarray(x[b].T.astype(np.float32, copy=False))
        for g in range(GROUPS):
            sl = slice(g * G, (g + 1) * G)
            in_maps.append(
                {
                    "xT": xT_b,
                    "wqT": np.ascontiguousarray(Wq[sl, :].T, dtype=np.float32),
                    "wkT": np.ascontiguousarray(Wk[sl, :].T, dtype=np.float32),
                    "wvT": np.ascontiguousarray(Wv[sl, :].T, dtype=np.float32),
                    "wpT": np.ascontiguousarray(Wp[:, sl].T, dtype=np.float32),
                    "bq": np.ascontiguousarray(bq[sl], dtype=np.float32),
                    "bk": np.ascontiguousarray(bk[sl], dtype=np.float32),
                    "bv": np.ascontiguousarray(bv[sl], dtype=np.float32),
                }
            )
    return in_maps


def run(inputs, trace=False):
    """Run on 8 cores; returns (out [B,T,C] fp32, BassKernelResults)."""
    x = np.asarray(inputs["x"], dtype=np.float32)
    T = x.shape[1]
    in_maps = _make_in_maps(
        x,
        np.asarray(inputs["Wq"]), np.asarray(inputs["bq"]),
        np.asarray(inputs["Wk"]), np.asarray(inputs["bk"]),
        np.asarray(inputs["Wv"]), np.asarray(inputs["bv"]),
        np.asarray(inputs["Wp"]),
    )
    nc = _get_nc(T)
    res = run_bass_kernel_spmd(
        nc, in_maps, core_ids=list(range(B * GROUPS)), trace=trace
    )
    bp = np.asarray(inputs["bp"], dtype=np.float32)
    parts = [res.results[i]["out"] for i in range(B * GROUPS)]
    out = np.stack(
        [sum(parts[b * GROUPS : (b + 1) * GROUPS]) for b in range(B)]
    ) + bp[None, None, :]
    return out.astype(np.float32), res


def kernel(**inputs):
    out, _ = run(inputs, trace=False)
    return out


# revision 3
# speedup vs baseline: 1.1788x; 1.1429x over previous
"""Multi-head self-attention (no mask) on 8 TRN2 NeuronCores.

Problem: B=2, T=2048, C=1024, H=16 heads, D=64.
    q/k/v = x @ W{q,k,v}.T + b;  att = softmax(q k^T / sqrt(D));
    y = att v;  out = y @ Wp.T + bp.

Sharding: core (b, g) with b in {0,1} batches x g in {0..3} head-groups of 4
heads.  Each core computes q/k/v for its 4 heads over the full sequence of its
batch, attention for those heads, and the partial output projection through its
256 columns of Wp.  The host sums the 4 partial projections per batch and adds
bp.  No device collectives needed.

v3 design (trace-driven):
  - All transposes on the HOST: kernel receives x^T [C,T], Wq/Wk/Wv^T [C,G],
    Wp^T [G,C] pre-transposed and pre-cast to bf16 (rel-err budget 2e-2,
    measured ~5e-3).  No PE transposes, no DVE drain copies.
  - ALL matmul operands bf16.  bf16 keeps the row-tiled concurrent S-pair
    (heads 2pi/2pi+1 at PE rows 0-63/64-127) at full rate (194ns/MM vs 500
    with fp32r operands sharing the moving XBUS), and avoids the ~2x
    slowdown observed for isolated fp32r matmuls inside a bf16 stream.
  - Loop order tq -> pi -> s.  Attention is ACT(exp)-bound (~1111ns per
    [128,1024] tile); the output projection for chunk tq-1 and the q^T
    projection for chunk tq+1 are fed one PE instruction per s-iteration
    into the idle PE slots (DVE/DMA steps of those pipelines flow freely).
  - Softmax denominators via a ones-column in each head's V (65-wide
    stationary); reciprocal on DVE, partition-broadcast on GPSIMD,
    normalize multiply on DVE (otherwise-idle engines).
  - DMA order: x chunk 0 + Wk/Wv first so the k-projection starts ~5us in.
"""

import sys
from collections import deque
from contextlib import ExitStack

import ml_dtypes
import numpy as np

if "/opt/trn_rl_repo" not in sys.path:
    sys.path.insert(0, "/opt/trn_rl_repo")

import concourse.bass as bass
import concourse.mybir as mybir
import concourse.tile as tile
from concourse import bacc
from concourse.bass_utils import run_bass_kernel_spmd

F32 = mybir.dt.float32
BF16 = mybir.dt.bfloat16
Act = mybir.ActivationFunctionType
BNP = ml_dtypes.bfloat16

P = 128
B, C, HEADS, D = 2, 1024, 16, 64
GROUPS = 4              # head groups (tensor-parallel dimension)
HLOC = HEADS // GROUPS  # 4 heads per core
G = HLOC * D            # 256 channels per core
KT = C // P             # 8 contraction tiles
VW = D + 1              # v group width incl. ones column


def build(T=2048):
    TQ = 512            # query-chunk (matmul free dim)
    NTQ = T // TQ
    NS = T // P         # key tiles

    nc = bacc.Bacc("TRN2", target_bir_lowering=False, debug=False)
    xT = nc.dram_tensor("xT", [C, T], BF16, kind="ExternalInput")
    wqT = nc.dram_tensor("wqT", [C, G], BF16, kind="ExternalInput")
    wkT = nc.dram_tensor("wkT", [C, G], BF16, kind="ExternalInput")
    wvT = nc.dram_tensor("wvT", [C, G], BF16, kind="ExternalInput")
    wpT = nc.dram_tensor("wpT", [G, C], BF16, kind="ExternalInput")
    bq = nc.dram_tensor("bq", [G], F32, kind="ExternalInput")
    bk = nc.dram_tensor("bk", [G], F32, kind="ExternalInput")
    bv = nc.dram_tensor("bv", [G], F32, kind="ExternalInput")
    out = nc.dram_tensor("out", [T, C], F32, kind="ExternalOutput")

    with tile.TileContext(nc) as tc, ExitStack() as ctx:
        persist = ctx.enter_context(tc.tile_pool(name="persist", bufs=1))

        ones4 = persist.tile([P, HLOC, 1], BF16, tag="ones4")
        nc.gpsimd.memset(ones4[:], 1.0)

        bq_pp = persist.tile([P, 2], F32, tag="bq_pp")
        bk_pp = persist.tile([P, 2], F32, tag="bk_pp")
        bv_row = persist.tile([1, G], F32, tag="bv_row")
        bv_bc = persist.tile([P, G], F32, tag="bv_bc")

        # persistent SBUF operands
        x_sb = persist.tile([P, KT, T], BF16, tag="x_sb")
        wq_sb = persist.tile([P, KT, G], BF16, tag="wq_sb")
        wk_sb = persist.tile([P, KT, G], BF16, tag="wk_sb")
        wv_sb = persist.tile([P, KT, G], BF16, tag="wv_sb")
        wp_sb = persist.tile([P, 2, C], BF16, tag="wp_sb")

        # DMA issue order = transfer order on the queue: smalls, then the
        # operands of the first compute (x chunk 0, Wk, Wv), then the rest.
        nc.sync.dma_start(bq_pp[:], bq[:].rearrange("(m p) -> p m", p=P))
        nc.sync.dma_start(bk_pp[:], bk[:].rearrange("(m p) -> p m", p=P))
        nc.sync.dma_start(bv_row[:], bv[None, :])
        nc.gpsimd.partition_broadcast(bv_bc[:, :], bv_row[0:1, :], channels=P)
        nc.sync.dma_start(
            x_sb[:, :, 0:TQ], xT[:, 0:TQ].rearrange("(a p) t -> p a t", p=P)
        )
        nc.sync.dma_start(wk_sb[:], wkT[:, :].rearrange("(a p) g -> p a g", p=P))
        nc.sync.dma_start(wv_sb[:], wvT[:, :].rearrange("(a p) g -> p a g", p=P))
        for c in range(1, NTQ):
            cs = slice(c * TQ, (c + 1) * TQ)
            nc.sync.dma_start(
                x_sb[:, :, cs], xT[:, cs].rearrange("(a p) t -> p a t", p=P)
            )
        nc.sync.dma_start(wq_sb[:], wqT[:, :].rearrange("(a p) g -> p a g", p=P))
        nc.sync.dma_start(wp_sb[:], wpT[:, :].rearrange("(j p) c -> p j c", p=P))

        qT = persist.tile([P, 2, T], BF16, tag="qT")
        kT = persist.tile([P, 2, T], BF16, tag="kT")
        v_sb = persist.tile([P, NS, HLOC * VW], BF16, tag="v_sb")
        yT = persist.tile([P, 2, T], BF16, tag="yT")

        # ---------------- phase A: k/v/q0 projections ----------------
        with tc.tile_pool(name="pa", bufs=2, space="PSUM") as pa:
            for c in range(NTQ):
                cs = slice(c * TQ, (c + 1) * TQ)
                # k^T chunk c: [G, TQ] channel-on-partition
                for m in range(2):
                    pk = pa.tile([P, TQ], F32, tag="pk")
                    for kk in range(KT):
                        nc.tensor.matmul(
                            pk[:],
                            wk_sb[:, kk, m * P : (m + 1) * P],
                            x_sb[:, kk, cs],
                            start=(kk == 0),
                            stop=(kk == KT - 1),
                        )
                    nc.scalar.activation(
                        kT[:, m, cs], pk[:], Act.Identity,
                        bias=bk_pp[:, m : m + 1], scale=1.0,
                    )
                # v tiles for keys in chunk c, natural layout + ones column
                for s in range(4 * c, 4 * c + 4):
                    pv = pa.tile([P, G], F32, tag="pv")
                    for kk in range(KT):
                        nc.tensor.matmul(
                            pv[:],
                            x_sb[:, kk, s * P : (s + 1) * P],
                            wv_sb[:, kk, :],
                            start=(kk == 0),
                            stop=(kk == KT - 1),
                        )
                    vs = v_sb[:, s, :].rearrange("p (h e) -> p h e", e=VW)
                    nc.vector.tensor_tensor(
                        vs[:, :, 0:D],
                        pv[:].rearrange("p (h d) -> p h d", d=D),
                        bv_bc[:].rearrange("p (h d) -> p h d", d=D),
                        op=mybir.AluOpType.add,
                    )
                    nc.vector.tensor_copy(vs[:, :, D : D + 1], ones4[:])
            # q^T chunk 0
            for m in range(2):
                pq = pa.tile([P, TQ], F32, tag="pk")
                for kk in range(KT):
                    nc.tensor.matmul(
                        pq[:],
                        wq_sb[:, kk, m * P : (m + 1) * P],
                        x_sb[:, kk, 0:TQ],
                        start=(kk == 0),
                        stop=(kk == KT - 1),
                    )
                nc.scalar.activation(
                    qT[:, m, 0:TQ], pq[:], Act.Identity,
                    bias=bq_pp[:, m : m + 1], scale=1.0,
                )

        # ---------------- phase B: attention + interleaved projections ------
        with (
            tc.tile_pool(name="ptp", bufs=3) as ptp,
            tc.tile_pool(name="npool", bufs=2) as npool,
            tc.tile_pool(name="osb", bufs=2) as osb_pool,
            tc.tile_pool(name="sps", bufs=2, space="PSUM") as sps,
            tc.tile_pool(name="yps", bufs=1, space="PSUM") as yps,
            tc.tile_pool(name="xps", bufs=1, space="PSUM") as xps,
        ):
            def qnext_steps(tqn):
                """q^T projection for chunk tqn; ('pe'|'other', closure) steps."""
                tqs = slice(tqn * TQ, (tqn + 1) * TQ)
                for m in range(2):
                    pq = xps.tile([P, TQ], F32, tag="pq")
                    for kk in range(KT):
                        yield "pe", lambda m=m, kk=kk, pq=pq: nc.tensor.matmul(
                            pq[:],
                            wq_sb[:, kk, m * P : (m + 1) * P],
                            x_sb[:, kk, tqs],
                            start=(kk == 0),
                            stop=(kk == KT - 1),
                        )
                    yield "other", lambda m=m, pq=pq: nc.vector.tensor_scalar_add(
                        qT[:, m, tqs], pq[:], bq_pp[:, m : m + 1]
                    )

            def oproj_steps(tqp, tail=False):
                """output projection for query chunk tqp (4 row-tiles).

                In the tail (nothing left to overlap) the PSUM double-buffers
                across the pq/po tags and the two staging copies split across
                ACT and DVE so the chain pipelines.
                """
                for i, mi in enumerate(range(4 * tqp, 4 * tqp + 4)):
                    ob = osb_pool.tile([P, C], F32, tag="ob")
                    for n in range(2):
                        ptag = ("po", "pq")[(2 * i + n) % 2] if tail else "po"
                        po = xps.tile([P, 512], F32, tag=ptag)
                        for j in range(2):
                            yield "pe", lambda mi=mi, n=n, j=j, po=po: nc.tensor.matmul(
                                po[:],
                                yT[:, j, mi * P : (mi + 1) * P],
                                wp_sb[:, j, n * 512 : (n + 1) * 512],
                                start=(j == 0),
                                stop=(j == 1),
                            )
                        if tail and n == 0:
                            yield "other", lambda n=n, po=po, ob=ob: nc.scalar.activation(
                                ob[:, n * 512 : (n + 1) * 512], po[:], Act.Identity,
                                bias=0.0, scale=1.0,
                            )
                        else:
                            yield "other", lambda n=n, po=po, ob=ob: nc.vector.tensor_copy(
                                ob[:, n * 512 : (n + 1) * 512], po[:]
                            )
                    yield "other", lambda mi=mi, ob=ob: nc.sync.dma_start(
                        out[mi * P : (mi + 1) * P, :], ob[:]
                    )

            def pump(extras, npe):
                """Emit queued steps: up to npe PE steps, 'other' steps freely."""
                while extras:
                    kind, fn = extras[0]
                    if kind == "pe":
                        if npe == 0:
                            return
                        npe -= 1
                    extras.popleft()
                    fn()

            for tq in range(NTQ):
                tqs = slice(tq * TQ, (tq + 1) * TQ)
                extras = deque()
                if tq + 1 < NTQ:
                    extras.extend(qnext_steps(tq + 1))
                if tq > 0:
                    extras.extend(oproj_steps(tq - 1))
                for pi in range(2):
                    py0 = yps.tile([VW, TQ], F32, tag="py0")
                    py1 = yps.tile([VW, TQ], F32, tag="py1")
                    py = [py0, py1]
                    for s in range(NS):
                        sp = sps.tile([P, 2 * TQ], F32, tag="sp")
                        for hh in range(2):
                            bp_ = 64 * hh
                            nc.tensor.matmul(
                                sp[:, hh * TQ : (hh + 1) * TQ],
                                kT[bp_ : bp_ + 64, pi, s * P : (s + 1) * P],
                                qT[bp_ : bp_ + 64, pi, tqs],
                                start=True,
                                stop=True,
                            )
                        pt = ptp.tile([P, 2 * TQ], BF16, tag="pt")
                        nc.scalar.activation(
                            pt[:], sp[:], Act.Exp, scale=1.0 / np.sqrt(D)
                        )
                        for hh in range(2):
                            h = 2 * pi + hh
                            nc.tensor.matmul(
                                py[hh][:],
                                v_sb[:, s, h * VW : (h + 1) * VW],
                                pt[:, hh * TQ : (hh + 1) * TQ],
                                start=(s == 0),
                                stop=(s == NS - 1),
                            )
                        pump(extras, 1)
                    # normalize: y_h / sums_h (sums live in PSUM row 64; the
                    # DVE reciprocal and gpsimd broadcast need partition-0
                    # inputs, so: DVE copy (aligned) -> DMA partition-shift ->
                    # reciprocal at base 0 -> broadcast -> multiply)
                    for hh in range(2):
                        srow = npool.tile([VW, TQ], F32, tag=f"srow{hh}")
                        nc.vector.tensor_copy(srow[D : D + 1, :], py[hh][D : D + 1, :])
                        srow0 = npool.tile([1, TQ], F32, tag=f"srow0{hh}")
                        nc.sync.dma_start(srow0[:], srow[D : D + 1, :])
                        recip0 = npool.tile([1, TQ], F32, tag=f"recip0{hh}")
                        nc.vector.reciprocal_approx_fast(recip0[0:1, :], srow0[0:1, :])
                        bcast = npool.tile([D, TQ], F32, tag=f"bcast{hh}")
                        nc.gpsimd.partition_broadcast(
                            bcast[:, :], recip0[0:1, :], channels=D
                        )
                        if hh == 0:
                            nc.vector.tensor_mul(
                                yT[0:D, pi, tqs], py[hh][0:D, :], bcast[:, :]
                            )
                        else:
                            y_tmp = npool.tile([D, TQ], BF16, tag="y_tmp")
                            nc.vector.tensor_mul(
                                y_tmp[:], py[hh][0:D, :], bcast[:, :]
                            )
                            nc.sync.dma_start(yT[D : 2 * D, pi, tqs], y_tmp[:])
                pump(extras, 1 << 30)
            # output projection for the final chunk
            tail = deque(oproj_steps(NTQ - 1, tail=True))
            pump(tail, 1 << 30)

    nc.finalize()
    return nc


_NC_CACHE = {}


def _get_nc(T=2048):
    if T not in _NC_CACHE:
        _NC_CACHE[T] = build(T=T)
    return _NC_CACHE[T]


def _make_in_maps(x, Wq, bq, Wk, bk, Wv, bv, Wp):
    in_maps = []
    wqTs = [np.ascontiguousarray(Wq[g * G : (g + 1) * G, :].T).astype(BNP)
            for g in range(GROUPS)]
    wkTs = [np.ascontiguousarray(Wk[g * G : (g + 1) * G, :].T).astype(BNP)
            for g in range(GROUPS)]
    wvTs = [np.ascontiguousarray(Wv[g * G : (g + 1) * G, :].T).astype(BNP)
            for g in range(GROUPS)]
    wpTs = [np.ascontiguousarray(Wp[:, g * G : (g + 1) * G].T).astype(BNP)
            for g in range(GROUPS)]
    for b in range(B):
        xT_b = np.ascontiguousarray(x[b].T).astype(BNP)
        for g in range(GROUPS):
            sl = slice(g * G, (g + 1) * G)
            in_maps.append(
                {
                    "xT": xT_b,
                    "wqT": wqTs[g],
                    "wkT": wkTs[g],
                    "wvT": wvTs[g],
                    "wpT": wpTs[g],
                    "bq": np.ascontiguousarray(bq[sl], dtype=np.float32),
                    "bk": np.ascontiguousarray(bk[sl], dtype=np.float32),
                    "bv": np.ascontiguousarray(bv[sl], dtype=np.float32),
                }
            )
    return in_maps


def run(inputs, trace=False):
    """Run on 8 cores; returns (out [B,T,C] fp32, BassKernelResults)."""
    x = np.asarray(inputs["x"], dtype=np.float32)
    T = x.shape[1]
    in_maps = _make_in_maps(
        x,
        np.asarray(inputs["Wq"]), np.asarray(inputs["bq"]),
        np.asarray(inputs["Wk"]), np.asarray(inputs["bk"]),
        np.asarray(inputs["Wv"]), np.asarray(inputs["bv"]),
        np.asarray(inputs["Wp"]),
    )
    nc = _get_nc(T)
    res = run_bass_kernel_spmd(
        nc, in_maps, core_ids=list(range(B * GROUPS)), trace=trace
    )
    bp = np.asarray(inputs["bp"], dtype=np.float32)
    parts = [res.results[i]["out"] for i in range(B * GROUPS)]
    out = np.stack(
        [sum(parts[b * GROUPS : (b + 1) * GROUPS]) for b in range(B)]
    ) + bp[None, None, :]
    return out.astype(np.float32), res


def kernel(**inputs):
    out, _ = run(inputs, trace=False)
    return out


# revision 5
# speedup vs baseline: 1.3585x; 1.1524x over previous
"""Multi-head self-attention (no mask) on 8 TRN2 NeuronCores.

Problem: B=2, T=2048, C=1024, H=16 heads, D=64.
    q/k/v = x @ W{q,k,v}.T + b;  att = softmax(q k^T / sqrt(D));
    y = att v;  out = y @ Wp.T + bp.

Sharding: core (b, g) with b in {0,1} batches x g in {0..3} head-groups of 4
heads.  Each core computes q/k/v for its 4 heads over the full sequence of its
batch, attention for those heads, and the partial output projection through its
256 columns of Wp.  The host sums the 4 partial projections per batch and adds
bp.  No device collectives needed.

v4 design (trace-driven):
  - All transposes on the HOST: kernel receives x^T [C,T], Wq/Wk/Wv^T [C,G],
    Wp^T [G,C] pre-transposed and pre-cast to bf16 (rel-err budget 2e-2,
    measured ~6e-3).  No PE transposes, no DVE drain copies.
  - ALL matmul operands bf16: keeps the row-tiled concurrent S-pair (heads
    2pi/2pi+1 at PE rows 0-63/64-127) at full rate (194ns/MM vs 500 with
    fp32r sharing the moving XBUS) and avoids the ~2x penalty on isolated
    fp32r matmuls inside a bf16 stream.
  - One global software pipeline over all (tq, pi, s) iterations: the
    attention phase is co-bound by ACT exp (~1111ns/tile) and the PE; the
    P.V matmuls run L=4 iterations behind S/exp, so head-pair boundaries
    never drain the PE.  PSUM accumulators are copied to SBUF by the DVE
    immediately after the last P.V matmul, freeing the bank for the next
    head pair; the softmax normalization works from the SBUF copy.
  - The output projection for chunk tq-1 and the q^T projection for chunk
    tq+1 are fed one PE instruction per iteration into the PE's idle slots
    (their DVE/ACT/DMA steps flow freely); out-proj is gated until the
    normalize that produces its yT has been emitted.
  - Head DMAs split across both HWDGE queues (x chunks on sync, weights on
    scalar) so the first k-projection starts as early as possible.
"""

import sys
from collections import deque
from contextlib import ExitStack

import ml_dtypes
import numpy as np

if "/opt/trn_rl_repo" not in sys.path:
    sys.path.insert(0, "/opt/trn_rl_repo")

import concourse.bass as bass
import concourse.mybir as mybir
import concourse.tile as tile
from concourse import bacc
from concourse.bass_utils import run_bass_kernel_spmd

F32 = mybir.dt.float32
BF16 = mybir.dt.bfloat16
Act = mybir.ActivationFunctionType
BNP = ml_dtypes.bfloat16

P = 128
B, C, HEADS, D = 2, 1024, 16, 64
GROUPS = 4              # head groups (tensor-parallel dimension)
HLOC = HEADS // GROUPS  # 4 heads per core
G = HLOC * D            # 256 channels per core
KT = C // P             # 8 contraction tiles
VW = D + 1              # v group width incl. ones column


def build(T=2048):
    TQ = 512            # query-chunk (matmul free dim)
    NTQ = T // TQ
    NS = T // P         # key tiles
    L = 4               # P.V lag (iterations) in the global pipeline

    nc = bacc.Bacc("TRN2", target_bir_lowering=False, debug=False)
    xT = nc.dram_tensor("xT", [C, T], BF16, kind="ExternalInput")
    wqT = nc.dram_tensor("wqT", [C, G], BF16, kind="ExternalInput")
    wkT = nc.dram_tensor("wkT", [C, G], BF16, kind="ExternalInput")
    wvT = nc.dram_tensor("wvT", [C, G], BF16, kind="ExternalInput")
    wpT = nc.dram_tensor("wpT", [G, C], BF16, kind="ExternalInput")
    bq = nc.dram_tensor("bq", [G], F32, kind="ExternalInput")
    bk = nc.dram_tensor("bk", [G], F32, kind="ExternalInput")
    bv = nc.dram_tensor("bv", [G], F32, kind="ExternalInput")
    out = nc.dram_tensor("out", [T, C], F32, kind="ExternalOutput")

    with tile.TileContext(nc) as tc, ExitStack() as ctx:
        persist = ctx.enter_context(tc.tile_pool(name="persist", bufs=1))

        ones4 = persist.tile([P, HLOC, 1], BF16, tag="ones4")
        nc.gpsimd.memset(ones4[:], 1.0)

        bq_pp = persist.tile([P, 2], F32, tag="bq_pp")
        bk_pp = persist.tile([P, 2], F32, tag="bk_pp")
        bv_row = persist.tile([1, G], F32, tag="bv_row")
        bv_bc = persist.tile([P, G], F32, tag="bv_bc")

        x_sb = persist.tile([P, KT, T], BF16, tag="x_sb")
        wq_sb = persist.tile([P, KT, G], BF16, tag="wq_sb")
        wk_sb = persist.tile([P, KT, G], BF16, tag="wk_sb")
        wv_sb = persist.tile([P, KT, G], BF16, tag="wv_sb")
        wp_sb = persist.tile([P, 2, C], BF16, tag="wp_sb")

        # x chunks + biases on the sync HWDGE queue, weights on the scalar
        # queue: both transfer in parallel, so the first k-projection (x
        # chunk 0 + Wk) starts a few us in.
        nc.sync.dma_start(bq_pp[:], bq[:].rearrange("(m p) -> p m", p=P))
        nc.sync.dma_start(bk_pp[:], bk[:].rearrange("(m p) -> p m", p=P))
        nc.sync.dma_start(bv_row[:], bv[None, :])
        nc.gpsimd.partition_broadcast(bv_bc[:, :], bv_row[0:1, :], channels=P)
        nc.scalar.dma_start(wk_sb[:], wkT[:, :].rearrange("(a p) g -> p a g", p=P))
        nc.scalar.dma_start(wv_sb[:], wvT[:, :].rearrange("(a p) g -> p a g", p=P))
        nc.scalar.dma_start(wq_sb[:], wqT[:, :].rearrange("(a p) g -> p a g", p=P))
        nc.scalar.dma_start(wp_sb[:], wpT[:, :].rearrange("(j p) c -> p j c", p=P))
        for c in range(NTQ):
            cs = slice(c * TQ, (c + 1) * TQ)
            nc.sync.dma_start(
                x_sb[:, :, cs], xT[:, cs].rearrange("(a p) t -> p a t", p=P)
            )

        qT = persist.tile([P, 2, T], BF16, tag="qT")
        kT = persist.tile([P, 2, T], BF16, tag="kT")
        v_sb = persist.tile([P, NS, HLOC * VW], BF16, tag="v_sb")
        yT = persist.tile([P, 2, T], BF16, tag="yT")

        # ---------------- phase A: k/v/q0 projections ----------------
        with tc.tile_pool(name="pa", bufs=2, space="PSUM") as pa:
            for c in range(NTQ):
                cs = slice(c * TQ, (c + 1) * TQ)
                for m in range(2):
                    pk = pa.tile([P, TQ], F32, tag="pk")
                    for kk in range(KT):
                        nc.tensor.matmul(
                            pk[:],
                            wk_sb[:, kk, m * P : (m + 1) * P],
                            x_sb[:, kk, cs],
                            start=(kk == 0),
                            stop=(kk == KT - 1),
                        )
                    nc.scalar.activation(
                        kT[:, m, cs], pk[:], Act.Identity,
                        bias=bk_pp[:, m : m + 1], scale=1.0,
                    )
                for s in range(4 * c, 4 * c + 4):
                    pv = pa.tile([P, G], F32, tag="pv")
                    for kk in range(KT):
                        nc.tensor.matmul(
                            pv[:],
                            x_sb[:, kk, s * P : (s + 1) * P],
                            wv_sb[:, kk, :],
                            start=(kk == 0),
                            stop=(kk == KT - 1),
                        )
                    vs = v_sb[:, s, :].rearrange("p (h e) -> p h e", e=VW)
                    nc.vector.tensor_tensor(
                        vs[:, :, 0:D],
                        pv[:].rearrange("p (h d) -> p h d", d=D),
                        bv_bc[:].rearrange("p (h d) -> p h d", d=D),
                        op=mybir.AluOpType.add,
                    )
                    nc.vector.tensor_copy(vs[:, :, D : D + 1], ones4[:])
            for m in range(2):
                pq = pa.tile([P, TQ], F32, tag="pk")
                for kk in range(KT):
                    nc.tensor.matmul(
                        pq[:],
                        wq_sb[:, kk, m * P : (m + 1) * P],
                        x_sb[:, kk, 0:TQ],
                        start=(kk == 0),
                        stop=(kk == KT - 1),
                    )
                nc.scalar.activation(
                    qT[:, m, 0:TQ], pq[:], Act.Identity,
                    bias=bq_pp[:, m : m + 1], scale=1.0,
                )

        # ---------------- phase B: pipelined attention + projections --------
        with (
            tc.tile_pool(name="ptp", bufs=L + 2) as ptp,
            tc.tile_pool(name="npool", bufs=2) as npool,
            tc.tile_pool(name="osb", bufs=2) as osb_pool,
            tc.tile_pool(name="sps", bufs=2, space="PSUM") as sps,
            tc.tile_pool(name="yps", bufs=1, space="PSUM") as yps,
            tc.tile_pool(name="xps", bufs=2, space="PSUM") as xps,
        ):
            def qnext_steps(tqn):
                """q^T projection for chunk tqn; ('pe'|'other', closure) steps."""
                tqs = slice(tqn * TQ, (tqn + 1) * TQ)
                for m in range(2):
                    pq = xps.tile([P, TQ], F32, tag="px")
                    for kk in range(KT):
                        yield "pe", lambda m=m, kk=kk, pq=pq: nc.tensor.matmul(
                            pq[:],
                            wq_sb[:, kk, m * P : (m + 1) * P],
                            x_sb[:, kk, tqs],
                            start=(kk == 0),
                            stop=(kk == KT - 1),
                        )
                    yield "other", lambda m=m, pq=pq: nc.vector.tensor_scalar_add(
                        qT[:, m, tqs], pq[:], bq_pp[:, m : m + 1]
                    )

            def oproj_steps(tqp, tail=False):
                """output projection for query chunk tqp (4 row-tiles)."""
                for mi in range(4 * tqp, 4 * tqp + 4):
                    ob = osb_pool.tile([P, C], F32, tag="ob")
                    for n in range(2):
                        po = xps.tile([P, 512], F32, tag="px")
                        for j in range(2):
                            yield "pe", lambda mi=mi, n=n, j=j, po=po: nc.tensor.matmul(
                                po[:],
                                yT[:, j, mi * P : (mi + 1) * P],
                                wp_sb[:, j, n * 512 : (n + 1) * 512],
                                start=(j == 0),
                                stop=(j == 1),
                            )
                        if tail and n == 0:
                            # in the tail ACT is idle; split the two staging
                            # copies across ACT and DVE so the chain pipelines
                            yield "other", lambda n=n, po=po, ob=ob: nc.scalar.activation(
                                ob[:, n * 512 : (n + 1) * 512], po[:], Act.Identity,
                                bias=0.0, scale=1.0,
                            )
                        else:
                            yield "other", lambda n=n, po=po, ob=ob: nc.vector.tensor_copy(
                                ob[:, n * 512 : (n + 1) * 512], po[:]
                            )
                    yield "other", lambda mi=mi, ob=ob: nc.sync.dma_start(
                        out[mi * P : (mi + 1) * P, :], ob[:]
                    )

            def normalize(pi, tq, pys):
                """softmax-normalize from the SBUF copies into yT.

                Denominators sit at row 64; the DVE reciprocal and gpsimd
                broadcast need partition-0 inputs (they ignore AP partition
                offsets), so shift via SBUF->SBUF DMA first.
                """
                tqs = slice(tq * TQ, (tq + 1) * TQ)
                srow0 = [None, None]
                recip0 = [None, None]
                bcast = [None, None]
                for hh in range(2):
                    srow0[hh] = npool.tile([1, TQ], F32, tag=f"srow0{hh}", name=f"srow0{hh}")
                    nc.sync.dma_start(srow0[hh][:], pys[hh][D : D + 1, :])
                for hh in range(2):
                    recip0[hh] = npool.tile([1, TQ], F32, tag=f"recip0{hh}", name=f"recip0{hh}")
                    nc.vector.reciprocal_approx_fast(
                        recip0[hh][0:1, :], srow0[hh][0:1, :]
                    )
                for hh in range(2):
                    bcast[hh] = npool.tile([D, TQ], F32, tag=f"bcast{hh}", name=f"bcast{hh}")
                    nc.gpsimd.partition_broadcast(
                        bcast[hh][:, :], recip0[hh][0:1, :], channels=D
                    )
                nc.vector.tensor_mul(
                    yT[0:D, pi, tqs], pys[0][0:D, :], bcast[0][:, :]
                )
                y_tmp = npool.tile([D, TQ], BF16, tag="y_tmp")
                nc.vector.tensor_mul(y_tmp[:], pys[1][0:D, :], bcast[1][:, :])
                nc.sync.dma_start(yT[D : 2 * D, pi, tqs], y_tmp[:])

            def pump(extras, npe):
                while extras:
                    kind, fn = extras[0]
                    if kind == "pe":
                        if npe == 0:
                            return
                        npe -= 1
                    extras.popleft()
                    fn()

            iters = [
                (tq, pi, s)
                for tq in range(NTQ) for pi in range(2) for s in range(NS)
            ]
            NIT = len(iters)
            extras = deque()
            pending = []        # (ready_u, steps)
            py = [None, None]
            for u in range(NIT + L):
                if u < NIT:
                    tq, pi, s = iters[u]
                    if pi == 0 and s == 0:
                        if tq + 1 < NTQ:
                            pending.append((u, deque(qnext_steps(tq + 1))))
                        if tq > 0:
                            pending.append((u + L, deque(oproj_steps(tq - 1))))
                    tqs = slice(tq * TQ, (tq + 1) * TQ)
                    sp = sps.tile([P, 2 * TQ], F32, tag="sp")
                    for hh in range(2):
                        bp_ = 64 * hh
                        nc.tensor.matmul(
                            sp[:, hh * TQ : (hh + 1) * TQ],
                            kT[bp_ : bp_ + 64, pi, s * P : (s + 1) * P],
                            qT[bp_ : bp_ + 64, pi, tqs],
                            start=True,
                            stop=True,
                        )
                    pt = ptp.tile([P, 2 * TQ], BF16, tag="pt")
                    nc.scalar.activation(
                        pt[:], sp[:], Act.Exp, scale=1.0 / np.sqrt(D)
                    )
                    iters[u] = (tq, pi, s, pt)
                if u >= L:
                    tq2, pi2, s2, pt2 = iters[u - L]
                    if s2 == 0:
                        py[0] = yps.tile([VW, TQ], F32, tag="py0", name="py0")
                        py[1] = yps.tile([VW, TQ], F32, tag="py1", name="py1")
                    for hh in range(2):
                        h = 2 * pi2 + hh
                        nc.tensor.matmul(
                            py[hh][:],
                            v_sb[:, s2, h * VW : (h + 1) * VW],
                            pt2[:, hh * TQ : (hh + 1) * TQ],
                            start=(s2 == 0),
                            stop=(s2 == NS - 1),
                        )
                    if s2 == NS - 1:
                        # drain the accumulators to SBUF at once: frees the
                        # PSUM banks for the next head pair ~1 iter later
                        pys = [None, None]
                        for hh in range(2):
                            pys[hh] = npool.tile([VW, TQ], F32, tag=f"pys{hh}", name=f"pys{hh}")
                            nc.vector.tensor_copy(pys[hh][:], py[hh][:])
                        normalize(pi2, tq2, pys)
                # activate pending extra groups, pace PE steps
                while pending and pending[0][0] <= u:
                    extras.extend(pending.pop(0)[1])
                npe_left = sum(1 for k, _ in extras if k == "pe")
                slots_left = max(1, (32 - (u % 32)))
                pump(extras, 2 if npe_left > slots_left else 1)
            pump(extras, 1 << 30)
            # output projection for the final chunk
            tail = deque(oproj_steps(NTQ - 1, tail=True))
            pump(tail, 1 << 30)

    nc.finalize()
    return nc


_NC_CACHE = {}


def _get_nc(T=2048):
    if T not in _NC_CACHE:
        _NC_CACHE[T] = build(T=T)
    return _NC_CACHE[T]


def _make_in_maps(x, Wq, bq, Wk, bk, Wv, bv, Wp):
    in_maps = []
    wqTs = [np.ascontiguousarray(Wq[g * G : (g + 1) * G, :].T).astype(BNP)
            for g in range(GROUPS)]
    wkTs = [np.ascontiguousarray(Wk[g * G : (g + 1) * G, :].T).astype(BNP)
            for g in range(GROUPS)]
    wvTs = [np.ascontiguousarray(Wv[g * G : (g + 1) * G, :].T).astype(BNP)
            for g in range(GROUPS)]
    wpTs = [np.ascontiguousarray(Wp[:, g * G : (g + 1) * G].T).astype(BNP)
            for g in range(GROUPS)]
    for b in range(B):
        xT_b = np.ascontiguousarray(x[b].T).astype(BNP)
        for g in range(GROUPS):
            sl = slice(g * G, (g + 1) * G)
            in_maps.append(
                {
                    "xT": xT_b,
                    "wqT": wqTs[g],
                    "wkT": wkTs[g],
                    "wvT": wvTs[g],
                    "wpT": wpTs[g],
                    "bq": np.ascontiguousarray(bq[sl], dtype=np.float32),
                    "bk": np.ascontiguousarray(bk[sl], dtype=np.float32),
                    "bv": np.ascontiguousarray(bv[sl], dtype=np.float32),
                }
            )
    return in_maps


def run(inputs, trace=False):
    """Run on 8 cores; returns (out [B,T,C] fp32, BassKernelResults)."""
    x = np.asarray(inputs["x"], dtype=np.float32)
    T = x.shape[1]
    in_maps = _make_in_maps(
        x,
        np.asarray(inputs["Wq"]), np.asarray(inputs["bq"]),
        np.asarray(inputs["Wk"]), np.asarray(inputs["bk"]),
        np.asarray(inputs["Wv"]), np.asarray(inputs["bv"]),
        np.asarray(inputs["Wp"]),
    )
    nc = _get_nc(T)
    res = run_bass_kernel_spmd(
        nc, in_maps, core_ids=list(range(B * GROUPS)), trace=trace
    )
    bp = np.asarray(inputs["bp"], dtype=np.float32)
    parts = [res.results[i]["out"] for i in range(B * GROUPS)]
    out = np.stack(
        [sum(parts[b * GROUPS : (b + 1) * GROUPS]) for b in range(B)]
    ) + bp[None, None, :]
    return out.astype(np.float32), res


def kernel(**inputs):
    out, _ = run(inputs, trace=False)
    return out


# revision 7
# speedup vs baseline: 1.3907x; 1.0237x over previous
"""Multi-head self-attention (no mask) on 8 TRN2 NeuronCores.

Problem: B=2, T=2048, C=1024, H=16 heads, D=64.
    q/k/v = x @ W{q,k,v}.T + b;  att = softmax(q k^T / sqrt(D));
    y = att v;  out = y @ Wp.T + bp.

Sharding: core (b, g) with b in {0,1} batches x g in {0..3} head-groups of 4
heads.  Each core computes q/k/v for its 4 heads over the full sequence of its
batch, attention for those heads, and the partial output projection through its
256 columns of Wp.  The host sums the 4 partial projections per batch and adds
bp.  No device collectives needed.

v5 design (trace-driven):
  - All transposes on the HOST, and every operand is laid out on the host in
    the exact SBUF tile layout (partition-major, chunk-major for x), so each
    DMA moves one large contiguous segment per partition (~8KB descriptors
    instead of 1KB rows; the head stall was DMA-descriptor-rate-bound).
  - ALL matmul operands bf16 (rel-err budget 2e-2, measured ~6e-3): keeps
    the row-tiled concurrent S-pair (heads 2pi/2pi+1 at PE rows 0-63/64-127)
    at full rate, and avoids the ~2x penalty on isolated fp32r matmuls
    inside a bf16 stream.
  - One global software pipeline over all (tq, pi, s) iterations, co-bound
    by ACT exp (~1111ns/tile) and the PE; the P.V matmuls run L=4
    iterations behind S/exp so head-pair boundaries never drain the PE.
    PSUM accumulators are drained to SBUF by the DVE right after the last
    P.V matmul, freeing the banks for the next head pair.
  - Softmax denominators via a leading ones-column in each head's V
    (stationary [1|v]): the denominator lands on PSUM partition 0, where
    the DVE reciprocal and the gpsimd partition-broadcast can read it
    directly (both ignore AP partition offsets); normalized y reaches its
    yT rows via SBUF->SBUF DMA partition-shift.
  - The output projection for chunk tq-1 (gated 20 iterations into the next
    window so the normalize chain it depends on has executed) and the q^T
    projection for chunk tq+1 are fed one PE instruction per iteration into
    the PE's idle slots; their DVE/ACT/DMA steps flow freely.
  - Tail: the final out-projection splits its PSUM drain copies across ACT
    and DVE and its DMAs across both HWDGE queues.
"""

import sys
from collections import deque
from contextlib import ExitStack

import ml_dtypes
import numpy as np

if "/opt/trn_rl_repo" not in sys.path:
    sys.path.insert(0, "/opt/trn_rl_repo")

import concourse.bass as bass
import concourse.mybir as mybir
import concourse.tile as tile
from concourse import bacc
from concourse.bass_utils import run_bass_kernel_spmd

F32 = mybir.dt.float32
BF16 = mybir.dt.bfloat16
Act = mybir.ActivationFunctionType
BNP = ml_dtypes.bfloat16

P = 128
B, C, HEADS, D = 2, 1024, 16, 64
GROUPS = 4              # head groups (tensor-parallel dimension)
HLOC = HEADS // GROUPS  # 4 heads per core
G = HLOC * D            # 256 channels per core
KT = C // P             # 8 contraction tiles
VW = D + 1              # v group width incl. leading ones column
TQ = 512                # query chunk (matmul moving free dim)


def build(T=2048):
    NTQ = T // TQ
    NS = T // P         # key tiles
    L = 4               # P.V lag (iterations) in the global pipeline

    nc = bacc.Bacc("TRN2", target_bir_lowering=False, debug=False)
    # inputs arrive pre-transposed AND pre-tiled into SBUF layout:
    #   xh  [NTQ, P, KT, TQ] : xh[c, p, a, t] = x[c*TQ+t, a*P+p]
    #   wh  [P, KT, G]       : wh[p, a, g]    = W[g, a*P+p]   (for q/k/v)
    #   wph [P, 2, C]        : wph[p, j, c]   = Wp[c, j*P+p]
    xh = nc.dram_tensor("xh", [NTQ, P, KT, TQ], BF16, kind="ExternalInput")
    wqh = nc.dram_tensor("wqh", [P, KT, G], BF16, kind="ExternalInput")
    wkh = nc.dram_tensor("wkh", [P, KT, G], BF16, kind="ExternalInput")
    wvh = nc.dram_tensor("wvh", [P, KT, G], BF16, kind="ExternalInput")
    wph = nc.dram_tensor("wph", [P, 2, C], BF16, kind="ExternalInput")
    bq = nc.dram_tensor("bq", [G], F32, kind="ExternalInput")
    bk = nc.dram_tensor("bk", [G], F32, kind="ExternalInput")
    bv = nc.dram_tensor("bv", [G], F32, kind="ExternalInput")
    out = nc.dram_tensor("out", [T, C], F32, kind="ExternalOutput")

    with tile.TileContext(nc) as tc, ExitStack() as ctx:
        persist = ctx.enter_context(tc.tile_pool(name="persist", bufs=1))

        ones4 = persist.tile([P, HLOC, 1], BF16, tag="ones4")
        nc.gpsimd.memset(ones4[:], 1.0)

        bq_pp = persist.tile([P, 2], F32, tag="bq_pp")
        bk_pp = persist.tile([P, 2], F32, tag="bk_pp")
        bv_row = persist.tile([1, G], F32, tag="bv_row")
        bv_bc = persist.tile([P, G], F32, tag="bv_bc")

        x_sb = persist.tile([P, NTQ, KT, TQ], BF16, tag="x_sb")
        wq_sb = persist.tile([P, KT, G], BF16, tag="wq_sb")
        wk_sb = persist.tile([P, KT, G], BF16, tag="wk_sb")
        wv_sb = persist.tile([P, KT, G], BF16, tag="wv_sb")
        wp_sb = persist.tile([P, 2, C], BF16, tag="wp_sb")

        # x chunks + biases on the sync HWDGE queue, weights on the scalar
        # queue; the first k-projection needs only x chunk 0 + Wk.
        nc.sync.dma_start(bq_pp[:], bq[:].rearrange("(m p) -> p m", p=P))
        nc.sync.dma_start(bk_pp[:], bk[:].rearrange("(m p) -> p m", p=P))
        nc.sync.dma_start(bv_row[:], bv[None, :])
        nc.gpsimd.partition_broadcast(bv_bc[:, :], bv_row[0:1, :], channels=P)
        nc.scalar.dma_start(wk_sb[:], wkh[:, :, :])
        nc.scalar.dma_start(wv_sb[:], wvh[:, :, :])
        nc.scalar.dma_start(wq_sb[:], wqh[:, :, :])
        nc.scalar.dma_start(wp_sb[:], wph[:, :, :])
        for c in range(NTQ):
            nc.sync.dma_start(x_sb[:, c], xh[c])

        qT = persist.tile([P, 2, T], BF16, tag="qT")
        kT = persist.tile([P, 2, T], BF16, tag="kT")
        v_sb = persist.tile([P, NS, HLOC * VW], BF16, tag="v_sb")
        yT = persist.tile([P, 2, T], BF16, tag="yT")

        # ---------------- phase A: k/v/q0 projections ----------------
        with tc.tile_pool(name="pa", bufs=2, space="PSUM") as pa:
            for c in range(NTQ):
                cs = slice(c * TQ, (c + 1) * TQ)
                for m in range(2):
                    pk = pa.tile([P, TQ], F32, tag="pk")
                    for kk in range(KT):
                        nc.tensor.matmul(
                            pk[:],
                            wk_sb[:, kk, m * P : (m + 1) * P],
                            x_sb[:, c, kk, :],
                            start=(kk == 0),
                            stop=(kk == KT - 1),
                        )
                    nc.scalar.activation(
                        kT[:, m, cs], pk[:], Act.Identity,
                        bias=bk_pp[:, m : m + 1], scale=1.0,
                    )
                for s in range(4 * c, 4 * c + 4):
                    si = s % 4
                    pv = pa.tile([P, G], F32, tag="pv")
                    for kk in range(KT):
                        nc.tensor.matmul(
                            pv[:],
                            x_sb[:, c, kk, si * P : (si + 1) * P],
                            wv_sb[:, kk, :],
                            start=(kk == 0),
                            stop=(kk == KT - 1),
                        )
                    vs = v_sb[:, s, :].rearrange("p (h e) -> p h e", e=VW)
                    nc.vector.tensor_tensor(
                        vs[:, :, 1:VW],
                        pv[:].rearrange("p (h d) -> p h d", d=D),
                        bv_bc[:].rearrange("p (h d) -> p h d", d=D),
                        op=mybir.AluOpType.add,
                    )
                    nc.vector.tensor_copy(vs[:, :, 0:1], ones4[:])
            for m in range(2):
                pq = pa.tile([P, TQ], F32, tag="pk")
                for kk in range(KT):
                    nc.tensor.matmul(
                        pq[:],
                        wq_sb[:, kk, m * P : (m + 1) * P],
                        x_sb[:, 0, kk, :],
                        start=(kk == 0),
                        stop=(kk == KT - 1),
                    )
                nc.scalar.activation(
                    qT[:, m, 0:TQ], pq[:], Act.Identity,
                    bias=bq_pp[:, m : m + 1], scale=1.0,
                )

        # ---------------- phase B: pipelined attention + projections --------
        with (
            tc.tile_pool(name="ptp", bufs=L + 2) as ptp,
            tc.tile_pool(name="npool", bufs=2) as npool,
            tc.tile_pool(name="osb", bufs=2) as osb_pool,
            tc.tile_pool(name="sps", bufs=2, space="PSUM") as sps,
            tc.tile_pool(name="yps", bufs=1, space="PSUM") as yps,
            tc.tile_pool(name="xps", bufs=2, space="PSUM") as xps,
        ):
            def qnext_steps(tqn):
                """q^T projection for chunk tqn; ('pe'|'other', closure) steps."""
                tqs = slice(tqn * TQ, (tqn + 1) * TQ)
                for m in range(2):
                    pq = xps.tile([P, TQ], F32, tag="px")
                    for kk in range(KT):
                        yield "pe", lambda m=m, kk=kk, pq=pq: nc.tensor.matmul(
                            pq[:],
                            wq_sb[:, kk, m * P : (m + 1) * P],
                            x_sb[:, tqn, kk, :],
                            start=(kk == 0),
                            stop=(kk == KT - 1),
                        )
                    yield "other", lambda m=m, pq=pq: nc.vector.tensor_scalar_add(
                        qT[:, m, tqs], pq[:], bq_pp[:, m : m + 1]
                    )

            def oproj_steps(tqp, tail=False):
                """output projection for query chunk tqp (4 row-tiles)."""
                for mi in range(4 * tqp, 4 * tqp + 4):
                    ob = osb_pool.tile([P, C], F32, tag="ob")
                    for n in range(2):
                        po = xps.tile([P, 512], F32, tag="px")
                        for j in range(2):
                            yield "pe", lambda mi=mi, n=n, j=j, po=po: nc.tensor.matmul(
                                po[:],
                                yT[:, j, mi * P : (mi + 1) * P],
                                wp_sb[:, j, n * 512 : (n + 1) * 512],
                                start=(j == 0),
                                stop=(j == 1),
                            )
                        if tail and n == 0:
                            # in the tail ACT is idle; split the two staging
                            # copies across ACT and DVE so the chain pipelines
                            yield "other", lambda n=n, po=po, ob=ob: nc.scalar.activation(
                                ob[:, n * 512 : (n + 1) * 512], po[:], Act.Identity,
                                bias=0.0, scale=1.0,
                            )
                        else:
                            yield "other", lambda n=n, po=po, ob=ob: nc.vector.tensor_copy(
                                ob[:, n * 512 : (n + 1) * 512], po[:]
                            )
                    eng = nc.scalar if (tail and mi % 2) else nc.sync
                    yield "other", lambda mi=mi, ob=ob, eng=eng: eng.dma_start(
                        out[mi * P : (mi + 1) * P, :], ob[:]
                    )

            def normalize(pi, tq, srcs, tail=False):
                """softmax-normalize into yT.

                srcs[hh] rows: 0 = denominator, 1..64 = unnormalized y.  The
                DVE reciprocal and gpsimd broadcast read partition 0 directly
                (they ignore partition offsets anyway); the normalized rows
                reach their yT partitions via SBUF->SBUF DMA shift.
                """
                tqs = slice(tq * TQ, (tq + 1) * TQ)
                for hh in range(2):
                    src = srcs[hh]
                    recip0 = npool.tile([1, TQ], F32, tag=f"recip0{hh}",
                                        name=f"recip0{hh}")
                    nc.vector.reciprocal_approx_fast(recip0[0:1, :], src[0:1, :])
                    bcast = npool.tile([VW, TQ], F32, tag=f"bcast{hh}",
                                       name=f"bcast{hh}")
                    nc.gpsimd.partition_broadcast(
                        bcast[:, :], recip0[0:1, :], channels=VW
                    )
                    ytmp = npool.tile([VW, TQ], BF16, tag=f"ytmp{hh}",
                                      name=f"ytmp{hh}")
                    # DVE partition bases must be 32-aligned: multiply all 65
                    # rows (row 0 = denom*recip, unused); DMA below reads 1:VW
                    nc.vector.tensor_mul(
                        ytmp[0:VW, :], src[0:VW, :], bcast[0:VW, :]
                    )
                    eng = nc.scalar if tail else nc.sync
                    eng.dma_start(yT[hh * D : (hh + 1) * D, pi, tqs], ytmp[1:VW, :])

            def pump(extras, npe):
                while extras:
                    kind, fn = extras[0]
                    if kind == "pe":
                        if npe == 0:
                            return
                        npe -= 1
                    extras.popleft()
                    fn()

            iters = [
                (tq, pi, s)
                for tq in range(NTQ) for pi in range(2) for s in range(NS)
            ]
            NIT = len(iters)
            extras = deque()
            pending = []        # (ready_u, steps)
            py = [None, None]
            for u in range(NIT + L):
                if u < NIT:
                    tq, pi, s = iters[u]
                    if pi == 0 and s == 0:
                        if tq + 1 < NTQ:
                            pending.append((u, deque(qnext_steps(tq + 1))))
                        if tq > 0:
                            # past the normalize(tq-1, pi=1) chain's execution
                            pending.append((u + 20, deque(oproj_steps(tq - 1))))
                    tqs = slice(tq * TQ, (tq + 1) * TQ)
                    sp = sps.tile([P, 2 * TQ], F32, tag="sp")
                    for hh in range(2):
                        bp_ = 64 * hh
                        nc.tensor.matmul(
                            sp[:, hh * TQ : (hh + 1) * TQ],
                            kT[bp_ : bp_ + 64, pi, s * P : (s + 1) * P],
                            qT[bp_ : bp_ + 64, pi, tqs],
                            start=True,
                            stop=True,
                        )
                    pt = ptp.tile([P, 2 * TQ], BF16, tag="pt")
                    nc.scalar.activation(
                        pt[:], sp[:], Act.Exp, scale=1.0 / np.sqrt(D)
                    )
                    iters[u] = (tq, pi, s, pt)
                if u >= L:
                    tq2, pi2, s2, pt2 = iters[u - L]
                    last = u - L == NIT - 1
                    if s2 == 0:
                        py[0] = yps.tile([VW, TQ], F32, tag="py0", name="py0")
                        py[1] = yps.tile([VW, TQ], F32, tag="py1", name="py1")
                    for hh in range(2):
                        h = 2 * pi2 + hh
                        nc.tensor.matmul(
                            py[hh][:],
                            v_sb[:, s2, h * VW : (h + 1) * VW],
                            pt2[:, hh * TQ : (hh + 1) * TQ],
                            start=(s2 == 0),
                            stop=(s2 == NS - 1),
                        )
                    if s2 == NS - 1:
                        if last:
                            # no next head pair: normalize straight off PSUM
                            normalize(pi2, tq2, py, tail=True)
                        else:
                            # drain accumulators to SBUF, freeing the PSUM
                            # banks for the next head pair ~1 iter later
                            pys = [None, None]
                            for hh in range(2):
                                pys[hh] = npool.tile(
                                    [VW, TQ], F32,
                                    tag=f"pys{hh}", name=f"pys{hh}",
                                )
                                nc.vector.tensor_copy(pys[hh][:], py[hh][:])
                            normalize(pi2, tq2, pys)
                while pending and pending[0][0] <= u:
                    extras.extend(pending.pop(0)[1])
                npe_left = sum(1 for k, _ in extras if k == "pe")
                slots_left = max(1, (32 - (u % 32)))
                pump(extras, 2 if npe_left > slots_left else 1)
            pump(extras, 1 << 30)
            # output projection for the final chunk
            tail = deque(oproj_steps(NTQ - 1, tail=True))
            pump(tail, 1 << 30)

    nc.finalize()
    return nc


_NC_CACHE = {}


def _get_nc(T=2048):
    if T not in _NC_CACHE:
        _NC_CACHE[T] = build(T=T)
    return _NC_CACHE[T]


def _sbufify_w(W_slice_T):
    """[C, G] -> [P, KT, G] with wh[p, a, g] = W^T[a*P+p, g]."""
    return np.ascontiguousarray(
        W_slice_T.reshape(KT, P, -1).transpose(1, 0, 2)
    ).astype(BNP)


def _make_in_maps(x, Wq, bq, Wk, bk, Wv, bv, Wp):
    in_maps = []
    wqhs = [_sbufify_w(Wq[g * G : (g + 1) * G, :].T) for g in range(GROUPS)]
    wkhs = [_sbufify_w(Wk[g * G : (g + 1) * G, :].T) for g in range(GROUPS)]
    wvhs = [_sbufify_w(Wv[g * G : (g + 1) * G, :].T) for g in range(GROUPS)]
    # wph[p, j, c] = Wp[c, g*G + j*P + p]
    wphs = [
        np.ascontiguousarray(
            Wp[:, g * G : (g + 1) * G].T.reshape(2, P, C).transpose(1, 0, 2)
        ).astype(BNP)
        for g in range(GROUPS)
    ]
    for b in range(B):
        T = x.shape[1]
        # xh[c, p, a, t] = x[b][c*TQ+t, a*P+p]
        xh_b = np.ascontiguousarray(
            x[b].T.reshape(KT, P, T // TQ, TQ).transpose(2, 1, 0, 3)
        ).astype(BNP)
        for g in range(GROUPS):
            sl = slice(g * G, (g + 1) * G)
            in_maps.append(
                {
                    "xh": xh_b,
                    "wqh": wqhs[g],
                    "wkh": wkhs[g],
                    "wvh": wvhs[g],
                    "wph": wphs[g],
                    "bq": np.ascontiguousarray(bq[sl], dtype=np.float32),
                    "bk": np.ascontiguousarray(bk[sl], dtype=np.float32),
                    "bv": np.ascontiguousarray(bv[sl], dtype=np.float32),
                }
            )
    return in_maps


def run(inputs, trace=False):
    """Run on 8 cores; returns (out [B,T,C] fp32, BassKernelResults)."""
    x = np.asarray(inputs["x"], dtype=np.float32)
    T = x.shape[1]
    in_maps = _make_in_maps(
        x,
        np.asarray(inputs["Wq"]), np.asarray(inputs["bq"]),
        np.asarray(inputs["Wk"]), np.asarray(inputs["bk"]),
        np.asarray(inputs["Wv"]), np.asarray(inputs["bv"]),
        np.asarray(inputs["Wp"]),
    )
    nc = _get_nc(T)
    res = run_bass_kernel_spmd(
        nc, in_maps, core_ids=list(range(B * GROUPS)), trace=trace
    )
    bp = np.asarray(inputs["bp"], dtype=np.float32)
    parts = [res.results[i]["out"] for i in range(B * GROUPS)]
    out = np.stack(
        [sum(parts[b * GROUPS : (b + 1) * GROUPS]) for b in range(B)]
    ) + bp[None, None, :]
    return out.astype(np.float32), res


def kernel(**inputs):
    out, _ = run(inputs, trace=False)
    return out
